# revision 38
# baseline (speedup 1.0000x reference)
"""GPT self-attention layer (B=2, S=2048, D=1024, H=16, hd=64) on 8 TRN2 cores.

Sharding: data-parallel over batch (2) x tensor-parallel over heads (4 groups
of 4 heads). Core c handles batch b=c//4, head group g=c%4.

v2 pipeline (bf16 operands, f32 PSUM accumulation, ~5e-3 relative error):
  1. xt/weights arrive bf16 from the host shard step. xT tiles stream on the
     sync DMA queue, weights on the scalar (ACT) queue, so the first QKV
     matmul starts as soon as xT0+wq land (~5us).
  2. Per 512-token group g: QKV projections (bias via DVE tensor_scalar,
     bf16 out), then attention tile (pair 0, qt=g) interleaved so ACT exp
     overlaps QKV matmuls of the next group.
  3. Attention per head pair, scoresT orientation [k-part, q-free]:
     per k-chunk: 2 score matmuls (h01) into a [P,2,512] psum tile,
     one exp (scale 0.125) -> bf16 pT, causal diagonal masked by DVE
     multiply, 2 ctx matmuls accumulate [V|1].T @ pT into per-h01 psum
     (row 64 = softmax denominator). PE order s(k) s(k+1) c(k) ... hides
     exp latency. Normalize: rden = 1/den via DVE divide (bf16), broadcast
     across 64 partitions via a 1-row outer-product matmul, DVE multiply
     into ctxn [128, 512] (both heads stacked).
  4. AllToAll per pair (bf16, 1MB) across all 8 cores; shards duplicated
     across batch halves so the program is core-independent. Pair-0 A2A
     overlaps pair-1 attention; its dma_gather is issued on gpsimd BETWEEN
     the two collectives so the even half of the output projection runs
     during A2A #2.
  5. out = ctxT_full.T @ Wo + bo, even (pair-0) dim chunks first, odd after
     A2A #2 lands, DMA to y [512, 1024] f32.

build(stage): stage in {"proj", "attn", "a2a", "full"} for bisection; partial
stages write debug data to y instead of the final output.
"""

import contextlib
import ctypes
import sys
import types

sys.path.insert(0, "/opt/trn_rl_repo")

import numpy as np
import ml_dtypes

import concourse.bass as bass
import concourse.mybir as mybir
import concourse.tile as tile
from concourse import bacc
from concourse import bass_utils

P = 128
B, S, D = 2, 2048, 1024
NH_LOC = 4          # heads per core
HD = 64             # head dim
G = NH_LOC * HD     # local head dims = 256
MC = G // P         # m-chunks of local dims = 2
DC = D // P         # d-chunks = 8
TB = 512            # token block (output tokens per core, q-tile width)
NQT = S // TB       # q-tiles = 4
NTC = S // P        # token chunks = 16
NC = 8

F32 = mybir.dt.float32
F32R = mybir.dt.float32r
BF16 = mybir.dt.bfloat16
I16 = mybir.dt.int16
Exp = mybir.ActivationFunctionType.Exp
MULT = mybir.AluOpType.mult
ADD = mybir.AluOpType.add

_STAGES = {"proj": 1, "attn": 2, "a2a": 3, "full": 4}

BF = ml_dtypes.bfloat16


def _install_ntff_hook():
    """Make trace=True work under axon: inject antenv.axon_hooks backed by
    ctypes calls into libaxon_pjrt.so (mirrors trn_agent_boot logic)."""
    if "antenv.axon_hooks" in sys.modules:
        return
    holder = {}
    mod = types.ModuleType("antenv.axon_hooks")
    mod.set_axon_ntff_profile_hook = lambda h: holder.update(h=h)
    mod.get_axon_ntff_profile_hook = lambda: holder.get("h")
    sys.modules["antenv.axon_hooks"] = mod
    try:
        lib = ctypes.CDLL("/opt/axon/libaxon_pjrt.so")
        if not hasattr(lib, "axon_start_nrt_profile"):
            return
    except OSError:
        return
    lib.axon_start_nrt_profile.argtypes = [
        ctypes.POINTER(ctypes.c_int64),
        ctypes.c_size_t,
    ]
    lib.axon_start_nrt_profile.restype = ctypes.c_int64
    lib.axon_stop_nrt_profile.argtypes = [ctypes.c_char_p]
    lib.axon_stop_nrt_profile.restype = ctypes.c_int64

    @contextlib.contextmanager
    def _hook(output_dir, device_ids):
        import jax

        jax.devices()
        if device_ids:
            ids = (ctypes.c_int64 * len(device_ids))(*device_ids)
            rc = lib.axon_start_nrt_profile(ids, len(device_ids))
        else:
            rc = lib.axon_start_nrt_profile(None, 0)
        if rc != 0:
            raise RuntimeError(f"axon_start_nrt_profile rc={rc}")
        try:
            yield
        finally:
            n = lib.axon_stop_nrt_profile(str(output_dir).encode())
            print(f"profile: {n} ntff file(s) written to {output_dir}")

    holder["h"] = _hook


def build(stage="full", coll=True, direct_norm=False):
    st = _STAGES[stage]
    nc = bacc.Bacc("TRN2", target_bir_lowering=False, debug=False, num_devices=NC)

    xt_d = nc.dram_tensor("xt", [D, S], BF16, kind="ExternalInput").ap()
    wq_d = nc.dram_tensor("wq", [D, G], BF16, kind="ExternalInput").ap()
    wk_d = nc.dram_tensor("wk", [D, G], BF16, kind="ExternalInput").ap()
    wv_d = nc.dram_tensor("wv", [D, G], BF16, kind="ExternalInput").ap()
    bq_d = nc.dram_tensor("bq", [P, MC], F32, kind="ExternalInput").ap()
    bk_d = nc.dram_tensor("bk", [P, MC], F32, kind="ExternalInput").ap()
    bv_d = nc.dram_tensor("bv", [1, G], F32, kind="ExternalInput").ap()
    wo_d = nc.dram_tensor("wo", [D, D], BF16, kind="ExternalInput").ap()
    bo_d = nc.dram_tensor("bo", [1, D], F32, kind="ExternalInput").ap()
    ma_d = nc.dram_tensor("ma", [P, 1], F32, kind="ExternalInput").ap()
    mb_d = nc.dram_tensor("mb", [P, 1], F32, kind="ExternalInput").ap()
    y_d = nc.dram_tensor("y", [TB, D], BF16, kind="ExternalOutput").ap()

    with tile.TileContext(nc) as tc:
        with (
            tc.tile_pool(name="const", bufs=1) as const,
            tc.tile_pool(name="dram", bufs=1, space="DRAM") as dram,
            tc.tile_pool(name="ps_pj", bufs=1, space="PSUM") as ps_pj,
            tc.tile_pool(name="ps_s", bufs=2, space="PSUM") as ps_s,
            tc.tile_pool(name="ps_c", bufs=3, space="PSUM") as ps_c,
            tc.tile_pool(name="persist", bufs=1) as persist,
        ):
            # ---------------- weight/bias DMAs on the scalar queue ----------
            wq_sb = persist.tile([P, DC, G], BF16, tag="wq")
            wk_sb = persist.tile([P, DC, G], BF16, tag="wk")
            wv_sb = persist.tile([P, DC, G], BF16, tag="wv")
            bq_sb = const.tile([P, MC], F32, tag="bq")
            bk_sb = const.tile([P, MC], F32, tag="bk")
            bv_row = const.tile([1, G], F32, tag="bv_row")
            bo_row = const.tile([1, D], F32, tag="bo_row")
            wo_sb = persist.tile([P, DC, D], BF16, tag="wo")
            ma_sb = const.tile([P, 1], F32, tag="ma")
            mb_sb = const.tile([P, 1], F32, tag="mb")
            nc.scalar.dma_start(wq_sb[:], wq_d.rearrange("(dc p) m -> p dc m", p=P))
            nc.scalar.dma_start(wk_sb[:], wk_d.rearrange("(dc p) m -> p dc m", p=P))

            # xT split across both HWDGE queues, one DMA per d-chunk: each is
            # a single contiguous 4KB line per partition, and QKV chains
            # start as soon as chunk dc=0 lands.
            xT = persist.tile([P, DC, S], BF16, tag="xT")
            xt_r = xt_d.rearrange("(dc p) t -> p dc t", p=P)
            for dc in range(DC // 2):
                nc.sync.dma_start(xT[:, dc], xt_r[:, dc])
            for dc in range(DC // 2, DC):
                nc.scalar.dma_start(xT[:, dc], xt_r[:, dc])
            nc.scalar.dma_start(wv_sb[:], wv_d.rearrange("(dc p) m -> p dc m", p=P))
            nc.sync.dma_start(bq_sb[:], bq_d)
            nc.sync.dma_start(bk_sb[:], bk_d)
            nc.sync.dma_start(bv_row[:], bv_d)
            nc.sync.dma_start(bo_row[:], bo_d)
            nc.sync.dma_start(ma_sb[:], ma_d)
            nc.sync.dma_start(mb_sb[:], mb_d)
            nc.scalar.dma_start(wo_sb[:], wo_d.rearrange("(dc p) n -> p dc n", p=P))

            # ---------------- constants ----------------
            # trimask[k, u] = 1 if k <= u else 0 (keep where u - k >= 0)
            tri_f = const.tile([P, P], F32, tag="tri_f")
            nc.gpsimd.memset(tri_f[:], 1.0)
            nc.gpsimd.affine_select(
                out=tri_f[:],
                in_=tri_f[:],
                compare_op=mybir.AluOpType.is_ge,
                fill=0.0,
                base=0,
                pattern=[[1, P]],
                channel_multiplier=-1,
            )
            tri_b = const.tile([P, P], BF16, tag="tri_b")
            nc.vector.tensor_copy(tri_b[:], tri_f[:])
            ones_f = const.tile([P, 1], F32, tag="ones_f")
            nc.vector.memset(ones_f[:], 1.0)
            # onescol_b[p, j] = 1 iff p == 64 (stationary for den broadcast)
            zrow_f = const.tile([P, HD], F32, tag="zrow_f")
            nc.vector.memset(zrow_f[:], 0.0)
            nc.vector.memset(zrow_f[64:65, :], 1.0)
            onescol_b = const.tile([P, HD], BF16, tag="onescol_b")
            nc.vector.tensor_copy(onescol_b[:], zrow_f[:])
            bv_bc = const.tile([P, G], F32, tag="bv_bc")
            nc.gpsimd.partition_broadcast(bv_bc[:], bv_row[:])
            bo_bc = const.tile([P, D], F32, tag="bo_bc")
            nc.gpsimd.partition_broadcast(bo_bc[:], bo_row[:])

            # persistent activations
            qT = persist.tile([P, MC, S], BF16, tag="qT")
            kT = persist.tile([P, MC, S], BF16, tag="kT")
            v_sb = persist.tile([P, NTC, NH_LOC * (HD + 1)], BF16, tag="v")

            # ones columns of v (denominator trick): col 64 of each head block
            v_ones_ap = v_sb[:].rearrange("p t (h c) -> p t h c", c=HD + 1)[
                :, :, :, HD
            ]
            nc.vector.tensor_copy(
                v_ones_ap,
                ones_f[:, 0:1, None].to_broadcast((P, NTC, NH_LOC, 1)),
            )
            v_heads = v_sb[:].rearrange("p t (h c) -> p t h c", c=HD + 1)
            # rdenX[h01]: moving operand for the den broadcast matmul; only
            # row 64 is live, the rest stay zero.
            rdenX = [
                persist.tile([P, TB], BF16, tag=f"rdenX{i}", name=f"rdenX{i}")
                for i in range(2)
            ]
            for i in range(2):
                nc.vector.memset(rdenX[i][:], 0.0)

            a2a_in = [
                dram.tile([NC * P, TB], BF16, name=f"a2ain{p}", tag=f"a2ain{p}")
                for p in range(2)
            ]
            a2a_out = [
                dram.tile([NC * P, TB], BF16, name=f"a2aout{p}", tag=f"a2aout{p}")
                for p in range(2)
            ]

            # ctxn[pair][qt]: [128, 512] bf16, h0 on partitions 0:64, h1 64:128
            ctxn = [
                [
                    persist.tile([P, TB], BF16, tag=f"ctxn{p}_{q}", name=f"ctxn{p}_{q}")
                    for q in range(NQT)
                ]
                for p in range(2)
            ]
            ctxf = persist.tile([P, 2, NQT, TB], BF16, tag="ctxf")
            ctxf2x = [
                persist.tile(
                    [P, 2, NQT, TB], BF16, tag=f"ctxf2x{pr}", name=f"ctxf2x{pr}"
                )
                for pr in range(2)
            ]
            o_parts = [
                persist.tile([P, 512], F32, tag=f"opart{u}", name=f"opart{u}")
                for u in range(8)
            ]

            pTp = tc.alloc_tile_pool(name="pTp", bufs=4)
            smallp = tc.alloc_tile_pool(name="smallp", bufs=6)

            def qkv_group(g, mcs=(0, 1), do_v=True):
                for w_sb, b_sb, out_t in ((wq_sb, bq_sb, qT), (wk_sb, bk_sb, kT)):
                    for mc_i in mcs:
                        pj = ps_pj.tile([P, 512], F32, tag="pj")
                        for dc in range(DC):
                            nc.tensor.matmul(
                                pj[:],
                                w_sb[:, dc, mc_i * P : (mc_i + 1) * P],
                                xT[:, dc, g * TB : (g + 1) * TB],
                                start=(dc == 0),
                                stop=(dc == DC - 1),
                            )
                        nc.vector.tensor_scalar_add(
                            out_t[:, mc_i, g * TB : (g + 1) * TB],
                            pj[:],
                            b_sb[:, mc_i : mc_i + 1],
                        )
                if not do_v:
                    return
                for ti in range(4):
                    tc_i = 4 * g + ti
                    pv = ps_c.tile([P, G], F32, tag="c")
                    for dc in range(DC):
                        nc.tensor.matmul(
                            pv[:],
                            xT[:, dc, tc_i * P : (tc_i + 1) * P],
                            wv_sb[:, dc, :],
                            start=(dc == 0),
                            stop=(dc == DC - 1),
                        )
                    nc.vector.tensor_tensor(
                        v_heads[:, tc_i, :, 0:HD],
                        pv[:].rearrange("p (h c) -> p h c", c=HD),
                        bv_bc[:].rearrange("p (h c) -> p h c", c=HD),
                        ADD,
                    )

            def attn_tile(pair, qt):
                """Attention for (pair, qt): PE order s(0) s(1) c(0) s(2)...
                then normalize + a2a_in sends."""
                nkc = 4 * qt + 4
                c_ps = [
                    ps_c.tile([P, 512], F32, tag="c", name=f"cps{h01}")
                    for h01 in range(2)
                ]
                pend = []  # (kc, pT, coff) awaiting ctx matmuls

                def do_ctx(kc, pT, coff):
                    for h01 in range(2):
                        h = 2 * pair + h01
                        nc.tensor.matmul(
                            c_ps[h01][0 : HD + 1, coff:512],
                            v_heads[:, kc, h, :],
                            pT[:, h01, coff:512],
                            start=(kc == 0),
                            stop=(kc == nkc - 1),
                        )

                for kc in range(nkc):
                    j = kc - 4 * qt
                    coff = max(0, j) * P
                    s_ps = ps_s.tile([P, 2, 512], F32, tag="s")
                    for h01 in range(2):
                        pb = h01 * HD
                        nc.tensor.matmul(
                            s_ps[:, h01, coff:512],
                            kT[pb : pb + HD, pair, kc * P : (kc + 1) * P],
                            qT[
                                pb : pb + HD,
                                pair,
                                qt * TB + coff : (qt + 1) * TB,
                            ],
                            start=True,
                            stop=True,
                        )
                    pT = pTp.tile([P, 2, 512], BF16, tag="pT")
                    nc.scalar.activation(
                        pT[:, :, coff:512],
                        s_ps[:, :, coff:512],
                        Exp,
                        scale=0.125,
                    )
                    if j >= 0:
                        nc.vector.tensor_tensor(
                            pT[:, :, coff : coff + P],
                            pT[:, :, coff : coff + P],
                            tri_b[:, None, :].to_broadcast((P, 2, P)),
                            MULT,
                        )
                    pend.append((kc, pT, coff))
                    if len(pend) >= 2:
                        do_ctx(*pend.pop(0))
                while pend:
                    do_ctx(*pend.pop(0))

                # normalize: rden = 1/den (DVE), broadcast via 1-row outer
                # product, multiply into ctxn (h0 -> partitions 0:64, h1 64:)
                for h01 in range(2):
                    den_sb = smallp.tile([1, TB], F32, tag="den_sb")
                    nc.vector.tensor_copy(den_sb[:], c_ps[h01][64:65, :])
                    rden_f = smallp.tile([1, TB], F32, tag="rden_f")
                    nc.vector.reciprocal_approx_fast(rden_f[:], den_sb[:])
                    nc.vector.tensor_copy(rdenX[h01][64:65, :], rden_f[:])
                    b_ps = ps_c.tile([P, 512], F32, tag="c", name=f"bps{h01}")
                    nc.tensor.matmul(
                        b_ps[0:HD, :],
                        onescol_b[:],
                        rdenX[h01][:],
                        start=True,
                        stop=True,
                    )
                    bb = smallp.tile([HD, TB], F32, tag="bb")
                    nc.vector.tensor_copy(bb[:], b_ps[0:HD, :])
                    nc.vector.tensor_tensor(
                        ctxn[pair][qt][h01 * HD : (h01 + 1) * HD, :],
                        c_ps[h01][0:HD, :],
                        bb[:],
                        MULT,
                    )
                # A2A sends: destination block qt in both batch halves; the
                # cross-batch copy is zeroed via the per-core masks so the
                # receiver can just ADD the halves (static addresses, no
                # per-core gather).
                if st >= 3:
                    for sh, m_sb in ((qt, ma_sb), (qt + 4, mb_sb)):
                        cz = smallp.tile([P, TB], BF16, tag="cz")
                        nc.vector.tensor_scalar_mul(
                            cz[:], ctxn[pair][qt][:, :], m_sb[:]
                        )
                        nc.sync.dma_start(
                            a2a_in[pair][sh * P : (sh + 1) * P, :], cz[:]
                        )

            def out_proj(pr, u):
                tc_i, nt = u // 2, u % 2
                if pr == 0:
                    po = ps_s.tile([P, 512], F32, tag="s", name=f"po0_{u}")
                    for i, g2 in enumerate(range(NQT)):
                        nc.tensor.matmul(
                            po[:],
                            ctxf[:, 0, g2, tc_i * P : (tc_i + 1) * P],
                            wo_sb[:, 2 * g2, nt * 512 : (nt + 1) * 512],
                            start=(i == 0),
                            stop=(i == NQT - 1),
                        )
                    nc.vector.tensor_tensor(
                        o_parts[u][:],
                        po[:],
                        bo_bc[:, nt * 512 : (nt + 1) * 512],
                        ADD,
                    )
                else:
                    # two independent 2-deep chains for ILP on the critical
                    # post-A2A#2 path
                    poA = ps_s.tile([P, 512], F32, tag="s", name=f"poA_{u}")
                    poB = ps_c.tile([P, 512], F32, tag="c", name=f"poB_{u}")
                    for i, (po, g2s) in enumerate(((poA, (0, 1)), (poB, (2, 3)))):
                        for j, g2 in enumerate(g2s):
                            nc.tensor.matmul(
                                po[:],
                                ctxf[:, 1, g2, tc_i * P : (tc_i + 1) * P],
                                wo_sb[:, 2 * g2 + 1, nt * 512 : (nt + 1) * 512],
                                start=(j == 0),
                                stop=(j == 1),
                            )
                    t_sb = outp.tile([P, 512], F32, tag="tsb")
                    nc.vector.tensor_tensor(t_sb[:], poA[:], o_parts[u][:], ADD)
                    o_sb = outp.tile([P, 512], BF16, tag="osb")
                    nc.vector.tensor_tensor(o_sb[:], poB[:], t_sb[:], ADD)
                    nc.sync.dma_start(
                        y_d[
                            tc_i * P : (tc_i + 1) * P,
                            nt * 512 : (nt + 1) * 512,
                        ],
                        o_sb[:],
                    )

            # ---------------- main schedule ----------------
            for g in range(NQT - 1):
                qkv_group(g)
                if st >= 2:
                    attn_tile(0, g)
                    if g >= 1:
                        attn_tile(1, g - 1)
            # last group: pair-0 slices first so A2A#1 can fire early
            qkv_group(NQT - 1, mcs=(0,), do_v=True)
            if st >= 2:
                attn_tile(0, NQT - 1)
            qkv_group(NQT - 1, mcs=(1,), do_v=False)
            if st >= 2:
                attn_tile(1, NQT - 2)

            if st >= 3 and coll:
                nc.gpsimd.collective_compute(
                    "AllToAll",
                    mybir.AluOpType.bypass,
                    ins=[a2a_in[0].opt()],
                    outs=[a2a_out[0].opt()],
                    replica_groups=[list(range(NC))],
                )
            if st >= 2:
                attn_tile(1, NQT - 1)

            if st >= 3 and coll:
                nc.gpsimd.collective_compute(
                    "AllToAll",
                    mybir.AluOpType.bypass,
                    ins=[a2a_in[1].opt()],
                    outs=[a2a_out[1].opt()],
                    replica_groups=[list(range(NC))],
                )
            if st >= 3:
                # receive: static-address unshuffle of both batch halves on
                # the scalar HWDGE queue, then add them (cross-batch half is
                # zeros) - no gpsimd gather, no CC-queue serialization.
                gsrc = a2a_out if coll else a2a_in
                for pr in range(2):
                    for g2 in range(NQT):
                        for half in range(2):
                            nc.scalar.dma_start(
                                ctxf2x[pr][:, half, g2],
                                gsrc[pr][
                                    half * TB + g2 * P : half * TB + (g2 + 1) * P, :
                                ],
                            )
                        nc.vector.tensor_tensor(
                            ctxf[:, pr, g2],
                            ctxf2x[pr][:, 0, g2],
                            ctxf2x[pr][:, 1, g2],
                            ADD,
                        )

            if st >= 4:
                with tc.tile_pool(name="outp", bufs=3) as outp:
                    # wait floors keep the scheduler from hoisting these
                    # matmuls ahead of attention work on the in-order PE
                    # stream (they'd head-of-line block on the gathers).
                    with tc.tile_wait_until(1.0):
                        for u in range(8):
                            out_proj(0, u)
                    with tc.tile_wait_until(1.1):
                        for u in range(8):
                            out_proj(1, u)

            # ---------------- debug outputs for partial stages ----------
            if st == 1:
                with tc.tile_pool(name="dbg", bufs=2) as dbg:
                    for tc_i in range(TB // P):
                        d_sb = dbg.tile([P, D], BF16, tag="dbg")
                        nc.vector.tensor_copy(d_sb[:, 0:512], qT[:, 0, 0:512])
                        nc.vector.tensor_copy(d_sb[:, 512:768], kT[:, 0, 0:256])
                        nc.vector.tensor_copy(
                            d_sb[:, 768:1024],
                            v_sb[:].rearrange("p t c -> p (t c)")[:, 0:256],
                        )
                        nc.sync.dma_start(y_d[tc_i * P : (tc_i + 1) * P, :], d_sb[:])
            if st == 2:
                with tc.tile_pool(name="dbg2", bufs=2) as dbg2:
                    for pr in range(2):
                        for q in range(NQT):
                            d_sb = dbg2.tile([P, TB], BF16, tag="dbg2")
                            nc.vector.tensor_copy(d_sb[:], ctxn[pr][q][:, :])
                            out_ap = (
                                y_d[:, :]
                                .rearrange("a b -> (a b)")
                                .rearrange(
                                    "(pr q p t) -> pr q p t", pr=2, q=NQT, p=P
                                )[pr, q]
                            )
                            nc.sync.dma_start(out_ap, d_sb[:])
            if st == 3:
                with tc.tile_pool(name="dbg3", bufs=2) as dbg3:
                    for tc_i in range(TB // P):
                        d_sb = dbg3.tile([P, D], BF16, tag="dbg3")
                        for dc in range(DC):
                            nc.vector.tensor_copy(
                                d_sb[:, dc * P : (dc + 1) * P],
                                ctxf[:, dc % 2, dc // 2, tc_i * P : (tc_i + 1) * P],
                            )
                        nc.sync.dma_start(y_d[tc_i * P : (tc_i + 1) * P, :], d_sb[:])

            smallp.release()
            pTp.release()

    nc.compile()
    return nc


_NC_CACHE = {}


def _get_nc(stage="full"):
    if stage not in _NC_CACHE:
        _NC_CACHE[stage] = build(stage)
    return _NC_CACHE[stage]


def _make_in_maps(x, Wq, bq, Wk, bk, Wv, bv, Wo, bo):
    x = np.asarray(x, np.float32)
    Wq, Wk, Wv, Wo = (np.asarray(a, np.float32) for a in (Wq, Wk, Wv, Wo))
    bq, bk, bv, bo = (np.asarray(a, np.float32) for a in (bq, bk, bv, bo))
    wo_b = np.ascontiguousarray(Wo.astype(BF))
    in_maps = []
    for c in range(NC):
        b, g = c // 4, c % 4
        sl = slice(g * G, (g + 1) * G)
        in_maps.append(
            {
                "xt": np.ascontiguousarray(x[b].T.astype(BF)),
                "wq": np.ascontiguousarray(Wq[:, sl].astype(BF)),
                "wk": np.ascontiguousarray(Wk[:, sl].astype(BF)),
                "wv": np.ascontiguousarray(Wv[:, sl].astype(BF)),
                "bq": np.ascontiguousarray(bq[sl].reshape(MC, P).T),
                "bk": np.ascontiguousarray(bk[sl].reshape(MC, P).T),
                "bv": np.ascontiguousarray(bv[sl].reshape(1, G)),
                "wo": wo_b,
                "bo": np.ascontiguousarray(bo.reshape(1, D)),
                "ma": np.full((P, 1), 1.0 if b == 0 else 0.0, np.float32),
                "mb": np.full((P, 1), 0.0 if b == 0 else 1.0, np.float32),
            }
        )
    return in_maps


def run(inputs, trace=False, tmpdir=None, stage="full"):
    """Run on 8 cores; returns (output [2,2048,1024], BassKernelResults)."""
    if trace:
        _install_ntff_hook()
    nc = _get_nc(stage)
    in_maps = _make_in_maps(**inputs)
    res = bass_utils.run_bass_kernel_spmd(
        nc, in_maps, core_ids=list(range(NC)), trace=trace, tmpdir=tmpdir
    )
    out = np.empty((B, S, D), np.float32)
    for c in range(NC):
        b, g = c // 4, c % 4
        out[b, g * TB : (g + 1) * TB, :] = res.results[c]["y"]
    return out, res


def kernel(**inputs) -> np.ndarray:
    out, _ = run(inputs, trace=False)
    return out
